# revision 12
# baseline (speedup 1.0000x reference)
"""Trainium2 Bass kernel for nn_Lowpass: 2D DCT -> keep 15x15 low-freq block -> 2D IDCT.

The op collapses to out[b,c] = P @ x[b,c] @ P^T with P = Di[:, :15] @ D[:15, :]
(a fixed 32x32 projection), data-parallel over 8 NeuronCores (3072 images each).

v2 pipeline (fp16 end-to-end on device, fp32 PSUM accumulation):
  1. HBM->SBUF via dma_start_transpose (xbar): the contiguous fp16 pack
     [2048, 128] lands transposed as L[hl*32+w, n*8+ht]  (h = 4*ht + hl).
     Large contiguous descriptors -> near line-rate, vs 128B descriptors for
     a strided load.
  2. MM-A (full 128x128 array, K=128): block-diag(P^T) stationary contracts w.
     The rhs free-dim AP enumerates (nh, ht, l) with l = n&1 innermost so the
     PSUM column order pairs fp16 elements by image-parity.
  3. Scalar-engine eviction PSUM->SBUF with fp32->fp16 cast (E1).
  4. One DVE 32x32 block transpose of E1 *bitcast to int32* (pairs of fp16
     move as one element - half the DVE work). This lands ALL five h bits
     (plus 2 image bits) in the partition dim.
  5. MM-C (K=128): block-diag-over-image-bits constant contracts h.
  6. Eviction PSUM->SBUF fp16 split between scalar + vector engines; plain
     contiguous store (4KB/partition descriptors). The host undoes the
     deterministic layout permutation and upcasts to fp32.
"""

import numpy as np

N = 32
FRE = 15
NCORES = 8
IMG_TOTAL = 8192 * 3          # 24576 images of 32x32
PER_CORE = IMG_TOTAL // NCORES  # 3072
PACK = 256                    # images per pipeline iteration (512KB fp16)
NPACK = PER_CORE // PACK      # 12


def _install_tilefix():
    """This container's walrus build rejects instructions carrying >1 sem wait
    ("Too many sync wait commands" in setupSyncWait). Tile attaches all of an
    instruction's required waits to the instruction itself. Split: for any
    instruction with N>1 waits, hoist N-1 of them onto fresh same-engine nop
    instructions placed immediately before it (same blocking semantics, one
    wait per instruction). Same treatment for the kernel-tail drain."""
    from concourse import mybir, tile
    from concourse.vector_clock import ScopedClock, VectorClock

    if getattr(tile.TileContext, "_tilefix_installed", False):
        return

    orig_lower = tile.TileContext._lower_ordered_insts

    def _lower_split(self, postordered_blocks):
        nc = self.nc
        for insts in postordered_blocks.values():
            new = []
            for inst in insts:
                si = getattr(inst, "sync_info", None)
                ow = list(si.on_wait) if si is not None and si.on_wait else []
                if len(ow) > 1:
                    for w in ow[:-1]:
                        nop = mybir.InstNoOp(
                            name=nc.get_next_instruction_name(), ins=[], outs=[])
                        nop.engine = inst.engine
                        nop.sync_info = mybir.SyncInfo(
                            on_wait=[w], on_update=[])
                        new.append(nop)
                    inst.sync_info = mybir.SyncInfo(
                        on_wait=[ow[-1]], on_update=list(si.on_update))
                new.append(inst)
            insts[:] = new
        return orig_lower(self, postordered_blocks)

    def _drain_and_barrier_split(self, tick_clock, wait_clock):
        nc = self.nc
        gc = tick_clock.global_clock
        n = len(gc)
        for proc in range(n):
            t = gc[proc]
            if t <= 0:
                continue
            vec = [0] * n
            vec[proc] = t
            nop_inst = nc.sync.nop()
            wait_clock.add_sem_waits(
                nop_inst.ins, ScopedClock({None: VectorClock(vec)})
            )
        nc.sync.drain()
        nc.all_engine_barrier()
        assert self.sems is not None
        popped = nc._tile_sem_poison_stack.pop()
        assert popped is self._sem_poison
        nc.clear_and_free_semaphores(list(self.sems.allocated().values()))
        nc.all_engine_barrier()

    tile.TileContext._lower_ordered_insts = _lower_split
    tile.TileContext._drain_and_barrier = _drain_and_barrier_split
    tile.TileContext._tilefix_installed = True

    # NTFF profiling hooks don't exist in this container; make trace=True
    # degrade gracefully inside run_bass_kernel_spmd.
    import sys as _sys
    import types as _types
    if "antenv.axon_hooks" not in _sys.modules:
        m = _types.ModuleType("antenv.axon_hooks")
        m.get_axon_ntff_profile_hook = lambda: None
        _sys.modules["antenv.axon_hooks"] = m


def _p_matrix():
    i = np.arange(N)
    D = 2.0 * np.cos(np.pi * (2 * i[None, :] + 1) * i[:, None] / (2 * N))
    Di = np.linalg.inv(D)
    P = Di[:, :FRE] @ D[:FRE, :]        # float64 [32, 32]
    return P


def _const_mats(np_dtype=np.float16, t_direct=False):
    """Stationary operands for the two matmul rounds.

    pA[hl*32+w, hl'*32+u] = delta(hl,hl') * P[u,w]      (contracts w -> u)
    int32-pair transpose path:
      pC[hl*32+nhl*8+ht, nhl'*32+a] = delta(nhl,nhl') * P[a, 4*ht+hl]
    direct fp32-transpose path:
      pC2[hl*32+nh1*16+ht*2+l, nh1'*64+l'*32+a] = delta * P[a, 4*ht+hl]
    """
    P = _p_matrix()
    pA = np.zeros((128, 128))
    for hl in range(4):
        pA[hl * 32:(hl + 1) * 32, hl * 32:(hl + 1) * 32] = P.T
    pC = np.zeros((128, 128))
    hls = np.arange(4)[:, None, None]
    hts = np.arange(8)[None, :, None]
    avs = np.arange(32)[None, None, :]
    blk = P[avs, 4 * hts + hls]                     # [hl, ht, a]
    if t_direct:
        for nh1 in range(2):
            for l in range(2):
                pC[hls * 32 + nh1 * 16 + hts * 2 + l,
                   nh1 * 64 + l * 32 + avs] = blk
    else:
        for nhl in range(4):
            pC[hls * 32 + nhl * 8 + hts, nhl * 32 + avs] = blk
    return np.ascontiguousarray(pA, dtype=np_dtype), \
        np.ascontiguousarray(pC, dtype=np_dtype)


def _build_program(mm_dtype_name="float16", loop_reps=1, dma_only=False,
                   e2=("scalar", "vector")):
    from concourse import bass, tile
    from concourse import mybir

    F32 = mybir.dt.float32
    I32 = mybir.dt.int32
    DT = getattr(mybir.dt, mm_dtype_name)
    assert mybir.dt.size(DT) == 2, "device dtype must be 2-byte"

    nc = bass.Bass("TRN2", target_bir_lowering=False, debug=False,
                   num_devices=NCORES)
    x_ext = nc.dram_tensor("x", [PER_CORE, N, N], DT, kind="ExternalInput").ap()
    pa_ext = nc.dram_tensor("pconstA", [128, 128], DT,
                            kind="ExternalInput").ap()
    pc_ext = nc.dram_tensor("pconstC", [128, 128], DT,
                            kind="ExternalInput").ap()
    y_ext = nc.dram_tensor("y", [NPACK, 128, 2048], DT,
                           kind="ExternalOutput").ap()

    with tile.TileContext(nc) as tc:
        with tc.tile_pool(name="const", bufs=1) as cpool, \
             tc.tile_pool(name="xin", bufs=3) as xpool, \
             tc.tile_pool(name="e1", bufs=2) as epool, \
             tc.tile_pool(name="tmid", bufs=2) as tpool, \
             tc.tile_pool(name="yout", bufs=2) as ypool, \
             tc.tile_pool(name="psA", bufs=1, space="PSUM") as papool, \
             tc.tile_pool(name="psB", bufs=1, space="PSUM") as pbpool:

            pa_t = cpool.tile([128, 128], DT)
            pc_t = cpool.tile([128, 128], DT)
            nc.sync.dma_start(pa_t[:], pa_ext[:])
            nc.sync.dma_start(pc_t[:], pc_ext[:])

            def stage1(p):
                """load + MM-A + E1 + transpose -> returns T tile."""
                base = p * PACK
                # ---- 1. transposed load: L[hl*32+w, n*8+ht] ----
                L = xpool.tile([128, 2048], DT)
                src = x_ext[base:base + PACK].rearrange(
                    "n (ht r) w -> (n ht) (r w)", ht=8)
                nc.sync.dma_start_transpose(L[:], src)

                Lv = L[:].rearrange("q (nh l ht) -> q nh ht l", nh=128, l=2,
                                    ht=8)
                E1 = epool.tile([128, 2048], DT)
                T = tpool.tile([128, 2048], DT)
                for hf in range(2):
                    # ---- 2. MM-A: contract w (K=128 full array) ----
                    # rhs enumerated (nh, ht, l): psum col = nh*16 + ht*2 + l
                    pa_ps = papool.tile([128, 1024], F32, tag=f"psA{hf}")
                    pav = pa_ps[:].rearrange("q (nh ht l) -> q nh ht l",
                                             nh=64, ht=8, l=2)
                    for c in range(2):
                        nh0 = 64 * hf + 32 * c
                        nc.tensor.matmul(
                            pav[:, 32 * c:32 * (c + 1)],
                            pa_t[:],
                            Lv[:, nh0:nh0 + 32],
                            start=True, stop=True,
                        )
                    # ---- 3. evict to fp16 on the scalar engine ----
                    eh = E1[:, 1024 * hf:1024 * (hf + 1)]
                    nc.scalar.copy(eh, pa_ps[:])
                    # ---- 4. int32-pair 32x32 block transpose (DVE) ----
                    # T[hl*32 + nhl*8 + ht, nhh*64 + u*2 + l]
                    th = T[:, 1024 * hf:1024 * (hf + 1)]
                    nc.vector.transpose(th.bitcast(I32), eh.bitcast(I32))
                return T

            def stage2(p, T):
                """MM-C + eviction + store for an already-transposed pack."""
                Y = ypool.tile([128, 2048], DT)
                for hf in range(2):
                    # ---- 5. MM-C: contract h (K=128) ----
                    pb_ps = pbpool.tile([128, 1024], F32, tag=f"psB{hf}")
                    for c in range(2):
                        t0 = 1024 * hf + 512 * c
                        nc.tensor.matmul(
                            pb_ps[:, 512 * c:512 * (c + 1)],
                            pc_t[:],
                            T[:, t0:t0 + 512],
                            start=True, stop=True,
                        )
                    # ---- 6. evict half (engine per e2 config) ----
                    yh = Y[:, 1024 * hf:1024 * (hf + 1)]
                    eng = e2[hf]
                    if eng == "scalar":
                        nc.scalar.copy(yh, pb_ps[:])
                    elif eng == "vector":
                        nc.vector.tensor_copy(yh, pb_ps[:])
                    elif eng == "gpsimd":
                        nc.gpsimd.tensor_copy(yh, pb_ps[:])
                    else:
                        raise ValueError(eng)
                # ---- 7. plain contiguous store ----
                nc.scalar.dma_start(y_ext[p], Y[:])

            if dma_only:
                for p_rep in range(NPACK * loop_reps):
                    p = p_rep % NPACK
                    L = xpool.tile([128, 2048], DT)
                    src = x_ext[p * PACK:(p + 1) * PACK].rearrange(
                        "n (ht r) w -> (n ht) (r w)", ht=8)
                    nc.sync.dma_start_transpose(L[:], src)
                    nc.scalar.dma_start(y_ext[p], L[:])
                return nc

            # 2-stage software pipeline: emit stage1(k) before stage2(k-1)
            # so every engine's instruction stream interleaves adjacent packs
            # (PE does MM-A(k) while the DVE transpose of pack k-1 finishes,
            # instead of stalling for its own MM-C).
            pending = None
            for p_rep in range(NPACK * loop_reps):
                p = p_rep % NPACK
                T = stage1(p)
                if pending is not None:
                    stage2(*pending)
                pending = (p, T)
            stage2(*pending)

    return nc


T_DIRECT = False
E2_ENGINES = ("scalar", "vector")


def _make_in_maps(x_flat, mm_dtype_name="float16", t_direct=None):
    if t_direct is None:
        t_direct = T_DIRECT
    np_dt = np.float16 if mm_dtype_name == "float16" else None
    if np_dt is None:
        import ml_dtypes
        np_dt = ml_dtypes.bfloat16
    x16 = np.ascontiguousarray(x_flat, dtype=np_dt)
    pA, pC = _const_mats(np_dt, t_direct=t_direct)
    return [
        {"x": x16[i * PER_CORE:(i + 1) * PER_CORE],
         "pconstA": pA, "pconstC": pC}
        for i in range(NCORES)
    ]


def _unscramble(y_core, t_direct=None):
    """[NPACK, 128, 2048] device layout -> [PER_CORE, 32, 32] float32.

    int32-pair path:
      y[pk, nhl*32+a, nhh*64+b*2+l] = out[pk*256 + nhh*8 + nhl*2 + l, a, b]
    direct path:
      y[pk, nh1*64+l*32+a, hf*1024+jj*32+b]
        = out[pk*256 + hf*128 + jj*4 + nh1*2 + l, a, b]
    """
    if t_direct is None:
        t_direct = T_DIRECT
    if t_direct:
        y = np.asarray(y_core).reshape(NPACK, 2, 2, 32, 2, 32, 32)
        y = y.transpose(0, 4, 5, 1, 2, 3, 6)   # pk, hf, jj, nh1, l, a, b
    else:
        y = np.asarray(y_core).reshape(NPACK, 4, 32, 32, 32, 2)
        y = y.transpose(0, 3, 1, 5, 2, 4)       # pk, nhh, nhl, l, a, b
    return np.ascontiguousarray(y, dtype=np.float32).reshape(PER_CORE, N, N)


def _run(x_flat, trace=False, mm_dtype_name="float16"):
    from concourse.bass_utils import run_bass_kernel_spmd

    _install_tilefix()
    nc = _build_program(mm_dtype_name, e2=E2_ENGINES)
    in_maps = _make_in_maps(x_flat, mm_dtype_name)
    core_ids = list(range(NCORES))
    bkr = run_bass_kernel_spmd(nc, in_maps, core_ids, trace=trace)
    out = np.concatenate(
        [_unscramble(bkr.results[i]["y"]) for i in core_ids], axis=0)
    return out, bkr


def kernel(x):
    x = np.asarray(x, dtype=np.float32)
    x_flat = x.reshape(IMG_TOTAL, N, N)
    out, _ = _run(x_flat, trace=False)
    return out.reshape(x.shape).astype(np.float32)
